# revision 1
# baseline (speedup 1.0000x reference)
"""Trainium2 Bass kernel for a 2-layer GAT + pooling + MLP classifier (CancerGNN).

Strategy (8-core SPMD):
  - Nodes sharded contiguously across 8 cores; edges sharded by destination
    node so segment softmax/sum are device-local.
  - Phase 1 (replicated): h_ext = x @ [W1@Ad | W1 | W1@As] -> per-node table
    [alpha_dst(4) | h(256) | alpha_src(4)] in DRAM.
  - Phase 2: per 128-dst-node block, indirect-DMA gather source rows per edge,
    compute p = exp(leaky_relu(as+ad)) (no max-subtraction needed; exponents
    are small), scatter via one-hot matmul into PSUM accumulating
    [p*h | p] -> normalized GAT1 output -> ELU -> matmul by W2ext -> layer-2
    table shard.
  - AllGather layer-2 table across cores.
  - Phase 3: same message passing for layer 2 (1 head), plus segment mean/max
    pooling (mean via one-hot matmul, max via masked running max).
  - Phase 4: tiny AllReduces of pooled stats + replicated 3-layer MLP.

The edge structure (per-block slot counts) is baked into the traced program;
the same program runs on all 8 cores (SPMD), per-core data arrives via inputs.
"""

import numpy as np

import concourse.bass as bass
import concourse.mybir as mybir
import concourse.tile as tile
from concourse.bass_utils import run_bass_kernel_spmd

P = 128
NCORES = 8
NGRAPH = 8
F32 = mybir.dt.float32
BF16 = mybir.dt.bfloat16
I32 = mybir.dt.int32

DEBUG = False

# numeric config
SPLIT_WAITS = True
PHASES = 4
MM_SCATTER_F32R = False   # use float32r fast mode for the scatter matmuls
MM_DENSE_F32R = False     # float32r for the dense x@W matmul (N=264 >= 256)
PAD_NEG = -1.0e30
NEG_SLOPE = 0.2


# ----------------------------------------------------------------------------
# host-side preparation
# ----------------------------------------------------------------------------

def _host_prep(x, edge_index, batch, W1, a_src1, a_dst1, W2, a_src2, a_dst2):
    n, in_dim = x.shape
    heads, hid = a_src1.shape
    c1 = heads * hid                      # 256
    npc = n // NCORES                     # nodes per core
    nb = -(-npc // P)                     # blocks per core
    last_rows = npc - (nb - 1) * P        # rows in last block

    # ---- combined weight tables ----
    # layer-1 table row: [h (c1) | alpha_src (heads) | alpha_dst (heads)]
    As1 = np.zeros((c1, heads), np.float32)
    Ad1 = np.zeros((c1, heads), np.float32)
    for h in range(heads):
        As1[h * hid:(h + 1) * hid, h] = a_src1[h]
        Ad1[h * hid:(h + 1) * hid, h] = a_dst1[h]
    W1ext = np.concatenate([W1, W1 @ As1, W1 @ Ad1], axis=1)  # [in_dim, c1+2*heads]
    # layer-2 table row: [h2 (hid) | alpha_src2 (1) | alpha_dst2 (1)]
    W2ext = np.concatenate([W2, W2 @ a_src2[0][:, None], W2 @ a_dst2[0][:, None]],
                           axis=1)                            # [c1, hid+2]

    d1 = W1ext.shape[1]                   # 264
    d2 = W2ext.shape[1]                   # 66

    pad1 = np.zeros((1, d1), np.float32)
    pad1[0, c1:c1 + heads] = PAD_NEG      # alpha_src cols
    pad2 = np.zeros((1, d2), np.float32)
    pad2[0, hid:hid + 1] = PAD_NEG

    # ---- edges: add self loops, sort by dst ----
    src = np.concatenate([edge_index[0], np.arange(n, dtype=np.int64)]).astype(np.int64)
    dst = np.concatenate([edge_index[1], np.arange(n, dtype=np.int64)]).astype(np.int64)
    order = np.argsort(dst, kind="stable")
    src_s = src[order]
    dst_s = dst[order]

    core_lo = np.searchsorted(dst_s, np.arange(NCORES) * npc)
    core_hi = np.searchsorted(dst_s, (np.arange(NCORES) + 1) * npc)

    # block edge counts per (core, block)
    cnts = np.zeros((NCORES, nb), np.int64)
    blk_of = np.empty(len(dst_s), np.int64)
    for c in range(NCORES):
        lo, hi = core_lo[c], core_hi[c]
        blk = (dst_s[lo:hi] - c * npc) // P
        blk_of[lo:hi] = blk
        cnts[c] = np.bincount(blk, minlength=nb)
    jb = np.maximum(1, -(-cnts.max(axis=0) // P)).astype(np.int64)  # slots per block
    totj = int(jb.sum())
    joff = np.concatenate([[0], np.cumsum(jb)])[:nb]

    # per-core edge arrays, layout [128, TOTJ] (partition-major per block)
    a_srcidx = np.full((NCORES, P, totj), n, np.int32)   # pad -> pad row
    a_dstidx = np.full((NCORES, P, totj), n, np.int32)
    a_slot = np.zeros((NCORES, P, totj), np.float32)
    for c in range(NCORES):
        lo = core_lo[c]
        boundaries = np.concatenate([[0], np.cumsum(cnts[c])])
        for b in range(nb):
            e0, e1 = lo + boundaries[b], lo + boundaries[b + 1]
            cnt = e1 - e0
            j = int(jb[b])
            cap = j * P
            es = np.full(cap, n, np.int64)
            ed = np.full(cap, n, np.int64)
            sl = np.zeros(cap, np.float32)
            es[:cnt] = src_s[e0:e1]
            ed[:cnt] = dst_s[e0:e1]
            sl[:cnt] = (dst_s[e0:e1] - (c * npc + b * P)).astype(np.float32)
            co = joff[b]
            a_srcidx[c, :, co:co + j] = es.reshape(j, P).T
            a_dstidx[c, :, co:co + j] = ed.reshape(j, P).T
            a_slot[c, :, co:co + j] = sl.reshape(j, P).T

    # pooling one-hot / mask, [128, NB*NGRAPH]
    a_onehot = np.zeros((NCORES, P, nb * NGRAPH), np.float32)
    a_mask = np.full((NCORES, P, nb * NGRAPH), -3.0e38, np.float32)
    for c in range(NCORES):
        for b in range(nb):
            g0 = c * npc + b * P
            rows = min(P, npc - b * P)
            bt = batch[g0:g0 + rows]
            oh = (bt[:, None] == np.arange(NGRAPH)[None, :]).astype(np.float32)
            a_onehot[c, :rows, b * NGRAPH:(b + 1) * NGRAPH] = oh
            a_mask[c, :rows, b * NGRAPH:(b + 1) * NGRAPH] = np.where(oh > 0, 0.0, -3.0e38)

    iota = np.broadcast_to(np.arange(P, dtype=np.float32), (P, P)).copy()
    ident = np.eye(P, dtype=np.float32)

    meta = dict(n=n, in_dim=in_dim, heads=heads, hid=hid, c1=c1, d1=d1, d2=d2,
                npc=npc, nb=nb, last_rows=last_rows,
                jb=[int(v) for v in jb], joff=[int(v) for v in joff], totj=totj)

    import ml_dtypes
    bf = ml_dtypes.bfloat16
    shared = dict(xT=np.ascontiguousarray(x.T), W1ext=W1ext, W2ext=W2ext,
                  pad1=pad1, pad2=pad2, iota=iota, ident=ident)
    per_core = dict(eidx_src=a_srcidx, eidx_dst=a_dstidx, slot=a_slot,
                    onehot=a_onehot, mask=a_mask)
    return meta, shared, per_core


# ----------------------------------------------------------------------------
# device program
# ----------------------------------------------------------------------------

def _mmcast(ap, f32r):
    return ap.bitcast(mybir.dt.float32r) if f32r else ap


def _build_program(meta, b1, b2, cW1, cb1, cW2, cb2, cW3, cb3):
    n = meta["n"]; in_dim = meta["in_dim"]; heads = meta["heads"]
    hid = meta["hid"]; c1 = meta["c1"]; d1 = meta["d1"]; d2 = meta["d2"]
    npc = meta["npc"]; nb = meta["nb"]; last_rows = meta["last_rows"]
    jb = meta["jb"]; joff = meta["joff"]; totj = meta["totj"]
    g1 = d1                  # gather width layer 1 (full rows)
    g2w = d2                 # gather width layer 2 (full rows)

    nc = bass.Bass()

    # ---- I/O ----
    xT_in = nc.dram_tensor("xT", [in_dim, n], F32, kind="ExternalInput")
    w1e_in = nc.dram_tensor("W1ext", [in_dim, d1], F32, kind="ExternalInput")
    w2e_in = nc.dram_tensor("W2ext", [c1, d2], F32, kind="ExternalInput")
    pad1_in = nc.dram_tensor("pad1", [1, d1], F32, kind="ExternalInput")
    pad2_in = nc.dram_tensor("pad2", [1, d2], F32, kind="ExternalInput")
    iota_in = nc.dram_tensor("iota", [P, P], F32, kind="ExternalInput")
    ident_in = nc.dram_tensor("ident", [P, P], F32, kind="ExternalInput")
    esrc_in = nc.dram_tensor("eidx_src", [P, totj], I32, kind="ExternalInput")
    edst_in = nc.dram_tensor("eidx_dst", [P, totj], I32, kind="ExternalInput")
    slot_in = nc.dram_tensor("slot", [P, totj], F32, kind="ExternalInput")
    oh_in = nc.dram_tensor("onehot", [P, nb * NGRAPH], F32, kind="ExternalInput")
    mask_in = nc.dram_tensor("mask", [P, nb * NGRAPH], F32, kind="ExternalInput")
    b1t_in = nc.dram_tensor("b1t", [P, c1], F32, kind="ExternalInput")
    b2t_in = nc.dram_tensor("b2t", [P, hid + 1], F32, kind="ExternalInput")
    cw1_in = nc.dram_tensor("cW1", [2 * hid, hid], F32, kind="ExternalInput")
    cw2_in = nc.dram_tensor("cW2", [hid, 32], F32, kind="ExternalInput")
    cw3_in = nc.dram_tensor("cW3", [32, 2], F32, kind="ExternalInput")
    cb1t_in = nc.dram_tensor("cb1t", [NGRAPH, hid], F32, kind="ExternalInput")
    cb2t_in = nc.dram_tensor("cb2t", [NGRAPH, 32], F32, kind="ExternalInput")
    cb3t_in = nc.dram_tensor("cb3t", [NGRAPH, 2], F32, kind="ExternalInput")
    out_ext = nc.dram_tensor("out", [NGRAPH, 2], F32, kind="ExternalOutput")
    h2_full = nc.dram_tensor("h2_full", [n + 1, d2], F32, addr_space="Shared")
    ad2_full = nc.dram_tensor("ad2_full", [n + 1, 1], F32, addr_space="Shared")
    sumr_d = nc.dram_tensor("sum_r", [NGRAPH, hid + 1], F32, addr_space="Shared")
    maxr_d = nc.dram_tensor("max_r", [hid, NGRAPH], F32, addr_space="Shared")

    nfull = (n // P) * P
    ntail = n - nfull

    with tile.TileContext(nc) as tc:
        with (
            tc.tile_pool(name="const", bufs=1) as constp,
            tc.tile_pool(name="dram", bufs=1, space="DRAM") as dram,
        ):
            h_ext1 = dram.tile([n + 1, d1], F32)
            adT = dram.tile([n + 1, heads], F32)
            h2_shard = dram.tile([npc, d2], F32)
            ad2_shard = dram.tile([npc, 1], F32)

            # ---- constants ----
            w1e_raw = constp.tile([in_dim, d1], F32)
            nc.sync.dma_start(w1e_raw[:], w1e_in[:])
            w1e = constp.tile([in_dim, d1], F32)
            nc.vector.tensor_copy(w1e[:], w1e_raw[:])
            w2ea_raw = constp.tile([P, d2], F32)
            nc.sync.dma_start(w2ea_raw[:], w2e_in[0:P, :])
            w2ea = constp.tile([P, d2], F32)
            nc.vector.tensor_copy(w2ea[:], w2ea_raw[:])
            w2eb_raw = constp.tile([P, d2], F32)
            nc.sync.dma_start(w2eb_raw[:], w2e_in[P:2 * P, :])
            w2eb = constp.tile([P, d2], F32)
            nc.vector.tensor_copy(w2eb[:], w2eb_raw[:])
            iota_t = constp.tile([P, P], F32)
            nc.sync.dma_start(iota_t[:], iota_in[:])
            ident_raw = constp.tile([P, P], F32)
            nc.sync.dma_start(ident_raw[:], ident_in[:])
            ident_t = constp.tile([P, P], F32)
            nc.vector.tensor_copy(ident_t[:], ident_raw[:])
            b1t = constp.tile([P, c1], F32)
            nc.sync.dma_start(b1t[:], b1t_in[:])
            b2t = constp.tile([P, hid + 1], F32)
            nc.sync.dma_start(b2t[:], b2t_in[:])
            esrc_t = constp.tile([P, totj], I32)
            nc.sync.dma_start(esrc_t[:], esrc_in[:])
            edst_t = constp.tile([P, totj], I32)
            nc.sync.dma_start(edst_t[:], edst_in[:])
            slot_t = constp.tile([P, totj], F32)
            nc.sync.dma_start(slot_t[:], slot_in[:])
            oh_raw = constp.tile([P, nb * NGRAPH], F32)
            nc.sync.dma_start(oh_raw[:], oh_in[:])
            oh_t = constp.tile([P, nb * NGRAPH], F32)
            nc.vector.tensor_copy(oh_t[:], oh_raw[:])
            mask_t = constp.tile([P, nb * NGRAPH], F32)
            nc.sync.dma_start(mask_t[:], mask_in[:])

            # pooling accumulators
            acc_sum = constp.tile([NGRAPH, hid + 1], F32)
            nc.vector.memset(acc_sum[:], 0.0)
            acc_max = constp.tile([P, NGRAPH, hid], F32)
            nc.vector.memset(acc_max[:], -3.0e38)

            # ================= phase 1: h_ext1 = x @ W1ext =================
            with (
                nc.named_scope("p1"),
                tc.tile_pool(name="p1x", bufs=3) as p1x,
                tc.tile_pool(name="p1w", bufs=3) as p1w,
                tc.tile_pool(name="p1ps", bufs=3, space="PSUM") as p1ps,
            ):
                XCH = 8  # node-blocks per x-load
                nxb = n // P + (1 if ntail else 0)
                for i0 in range(0, nxb, XCH):
                    i1 = min(i0 + XCH, nxb)
                    cols0 = i0 * P
                    cols1 = min(i1 * P, n)
                    xload = p1x.tile([in_dim, XCH * P], F32, tag="xload")
                    nc.sync.dma_start(xload[:, :cols1 - cols0],
                                      xT_in[:, cols0:cols1])
                    wtile = p1w.tile([P, XCH, d1], F32, tag="wtile")
                    for q in range(i1 - i0):
                        m = min(P, n - (i0 + q) * P)
                        ps = p1ps.tile([P, d1], F32, tag="p1ps")
                        nc.tensor.matmul(
                            ps[:m],
                            lhsT=_mmcast(xload[:, q * P:q * P + m], MM_DENSE_F32R),
                            rhs=_mmcast(w1e[:], MM_DENSE_F32R),
                            start=True, stop=True,
                        )
                        nc.vector.tensor_copy(wtile[:m, q, :], ps[:m])
                    if cols1 - cols0 == XCH * P:
                        dst_ap = h_ext1[cols0:cols1, :].rearrange(
                            "(j p) c -> p j c", p=P)
                        nc.sync.dma_start(dst_ap, wtile[:])
                        ad_ap = adT[cols0:cols1, :].rearrange(
                            "(j p) c -> p j c", p=P)
                        nc.sync.dma_start(ad_ap, wtile[:, :, c1 + heads:])
                    else:
                        for q in range(i1 - i0):
                            m = min(P, n - (i0 + q) * P)
                            nc.sync.dma_start(
                                h_ext1[(i0 + q) * P:(i0 + q) * P + m, :],
                                wtile[:m, q, :])
                            nc.sync.dma_start(
                                adT[(i0 + q) * P:(i0 + q) * P + m, :],
                                wtile[:m, q, c1 + heads:])
                nc.sync.dma_start(h_ext1[n:n + 1, :], pad1_in[:])
                nc.sync.dma_start(adT[n:n + 1, :], pad1_in[:, 0:heads])

            # ================= phase 2: GAT layer 1 + W2ext =================
            with (
                nc.named_scope("p2"),
                tc.tile_pool(name="edge", bufs=3) as edgep,
                tc.tile_pool(name="blk", bufs=2) as blkp,
                tc.tile_pool(name="ps1", bufs=3, space="PSUM") as ps1p,
                tc.tile_pool(name="pst", bufs=2, space="PSUM") as pstp,
                tc.tile_pool(name="ps2", bufs=2, space="PSUM") as ps2p,
            ):
                for b in range(nb if PHASES >= 2 else 0):
                    j = jb[b]
                    co = joff[b]
                    rows = min(P, npc - b * P)
                    g = edgep.tile([P, j, g1], F32, tag="g1")
                    gB = edgep.tile([P, j, heads], F32, tag="gB1")
                    for q in range(j):
                        nc.gpsimd.indirect_dma_start(
                            out=g[:, q, :], out_offset=None,
                            in_=h_ext1[:],
                            in_offset=bass.IndirectOffsetOnAxis(
                                ap=esrc_t[:, co + q:co + q + 1], axis=0),
                        )
                        nc.gpsimd.indirect_dma_start(
                            out=gB[:, q, :], out_offset=None,
                            in_=adT[:],
                            in_offset=bass.IndirectOffsetOnAxis(
                                ap=edst_t[:, co + q:co + q + 1], axis=0),
                        )
                    al = g[:, :, c1:c1 + heads]
                    # e = leaky_relu(alpha_src + alpha_dst); p = exp(e)
                    nc.vector.tensor_tensor(out=al, in0=al, in1=gB[:],
                                            op=mybir.AluOpType.add)
                    nc.vector.tensor_scalar_mul(gB[:], al, NEG_SLOPE)
                    nc.vector.tensor_tensor(out=al, in0=al, in1=gB[:],
                                            op=mybir.AluOpType.max)
                    nc.vector.tensor_scalar_max(al, al, -60.0)
                    nc.scalar.activation(al, al, mybir.ActivationFunctionType.Exp)
                    # msg = [p*h | p]
                    nc.vector.tensor_tensor(
                        out=g[:, :, 0:c1].rearrange("p j (h c) -> p j h c", h=heads),
                        in0=g[:, :, 0:c1].rearrange("p j (h c) -> p j h c", h=heads),
                        in1=g[:, :, c1:c1 + heads][:, :, :, None].to_broadcast(
                            [P, j, heads, hid]),
                        op=mybir.AluOpType.mult,
                    )
                    # one-hot
                    s_t = edgep.tile([P, j, P], F32, tag="s1")
                    nc.vector.tensor_tensor(
                        out=s_t[:],
                        in0=slot_t[:, co:co + j].to_broadcast([P, j, P]),
                        in1=iota_t[:, None, :].to_broadcast([P, j, P]),
                        op=mybir.AluOpType.is_equal,
                    )
                    ps1 = ps1p.tile([P, d1 - heads], F32, tag="ps1")
                    for q in range(j):
                        nc.tensor.matmul(
                            ps1[:],
                            lhsT=_mmcast(s_t[:, q, :], MM_SCATTER_F32R),
                            rhs=_mmcast(g[:, q, 0:c1 + heads], MM_SCATTER_F32R),
                            start=(q == 0), stop=(q == j - 1),
                        )
                    # normalize + b1 + ELU
                    den = blkp.tile([P, heads], F32, tag="den1")
                    nc.vector.tensor_scalar_max(den[:], ps1[:, c1:c1 + heads], 1e-30)
                    rec = blkp.tile([P, heads], F32, tag="rec1")
                    nc.vector.reciprocal(rec[:], den[:])
                    h1 = blkp.tile([P, c1], F32, tag="h1")
                    nc.vector.tensor_tensor(
                        out=h1[:].rearrange("p (h c) -> p h c", h=heads),
                        in0=ps1[:, 0:c1].rearrange("p (h c) -> p h c", h=heads),
                        in1=rec[:, :, None].to_broadcast([P, heads, hid]),
                        op=mybir.AluOpType.mult,
                    )
                    nc.vector.tensor_tensor(out=h1[:], in0=h1[:], in1=b1t[:],
                                            op=mybir.AluOpType.add)
                    tmin = blkp.tile([P, c1], F32, tag="tmin")
                    nc.vector.tensor_scalar_min(tmin[:], h1[:], 0.0)
                    nc.scalar.activation(tmin[:], tmin[:],
                                         mybir.ActivationFunctionType.Exp)
                    nc.vector.tensor_scalar_max(h1[:], h1[:], 0.0)
                    nc.vector.tensor_tensor(out=h1[:], in0=h1[:], in1=tmin[:],
                                            op=mybir.AluOpType.add)
                    nc.vector.tensor_scalar_add(h1[:], h1[:], -1.0)
                    # h2 = h1' @ W2ext  (transpose h1' halves, 2 matmuls)
                    h1T = blkp.tile([P, 2, P], F32, tag="h1T")
                    for half in range(2):
                        tp = pstp.tile([P, P], F32, tag="tp")
                        nc.tensor.transpose(tp[:], h1[:, half * P:(half + 1) * P],
                                            ident_t[:])
                        nc.vector.tensor_copy(h1T[:, half, :], tp[:])
                    ps2 = ps2p.tile([P, d2], F32, tag="ps2")
                    nc.tensor.matmul(ps2[:], lhsT=_mmcast(h1T[:, 0, :], False),
                                     rhs=_mmcast(w2ea[:], False),
                                     start=True, stop=False)
                    nc.tensor.matmul(ps2[:], lhsT=_mmcast(h1T[:, 1, :], False),
                                     rhs=_mmcast(w2eb[:], False),
                                     start=False, stop=True)
                    h2row = blkp.tile([P, d2], F32, tag="h2row")
                    nc.vector.tensor_copy(h2row[:], ps2[:])
                    nc.sync.dma_start(h2_shard[b * P:b * P + rows, :], h2row[:rows])
                    nc.sync.dma_start(ad2_shard[b * P:b * P + rows, :],
                                      h2row[:rows, hid + 1:hid + 2])

            # ================= allgather layer-2 table =================
            with nc.named_scope("ag"):
                if PHASES >= 3:
                    nc.gpsimd.collective_compute(
                        "AllGather",
                        mybir.AluOpType.bypass,
                        replica_groups=[list(range(NCORES))],
                        ins=[h2_shard[:]],
                        outs=[h2_full[0:n, :]],
                    )
                    nc.gpsimd.collective_compute(
                        "AllGather",
                        mybir.AluOpType.bypass,
                        replica_groups=[list(range(NCORES))],
                        ins=[ad2_shard[:]],
                        outs=[ad2_full[0:n, :]],
                    )
                    nc.sync.dma_start(h2_full[n:n + 1, :], pad2_in[:])
                    nc.sync.dma_start(ad2_full[n:n + 1, :], pad1_in[:, 0:1])

            # ================= phase 3: GAT layer 2 + pooling =================
            with (
                nc.named_scope("p3"),
                tc.tile_pool(name="edge3", bufs=3) as edgep,
                tc.tile_pool(name="blk3", bufs=2) as blkp,
                tc.tile_pool(name="ps1_3", bufs=3, space="PSUM") as ps1p,
                tc.tile_pool(name="ps2_3", bufs=2, space="PSUM") as ps2p,
            ):
                for b in range(nb if PHASES >= 4 else 0):
                    j = jb[b]
                    co = joff[b]
                    g = edgep.tile([P, j, g2w], F32, tag="g2")
                    gB = edgep.tile([P, j, 1], F32, tag="gB2")
                    for q in range(j):
                        nc.gpsimd.indirect_dma_start(
                            out=g[:, q, :], out_offset=None,
                            in_=h2_full[:],
                            in_offset=bass.IndirectOffsetOnAxis(
                                ap=esrc_t[:, co + q:co + q + 1], axis=0),
                        )
                        nc.gpsimd.indirect_dma_start(
                            out=gB[:, q, :], out_offset=None,
                            in_=ad2_full[:],
                            in_offset=bass.IndirectOffsetOnAxis(
                                ap=edst_t[:, co + q:co + q + 1], axis=0),
                        )
                    al = g[:, :, hid:hid + 1]
                    nc.vector.tensor_tensor(out=al, in0=al, in1=gB[:],
                                            op=mybir.AluOpType.add)
                    nc.vector.tensor_scalar_mul(gB[:], al, NEG_SLOPE)
                    nc.vector.tensor_tensor(out=al, in0=al, in1=gB[:],
                                            op=mybir.AluOpType.max)
                    nc.vector.tensor_scalar_max(al, al, -60.0)
                    nc.scalar.activation(al, al, mybir.ActivationFunctionType.Exp)
                    nc.vector.tensor_tensor(
                        out=g[:, :, 0:hid],
                        in0=g[:, :, 0:hid],
                        in1=g[:, :, hid:hid + 1].to_broadcast([P, j, hid]),
                        op=mybir.AluOpType.mult,
                    )
                    s_t = edgep.tile([P, j, P], F32, tag="s2")
                    nc.vector.tensor_tensor(
                        out=s_t[:],
                        in0=slot_t[:, co:co + j].to_broadcast([P, j, P]),
                        in1=iota_t[:, None, :].to_broadcast([P, j, P]),
                        op=mybir.AluOpType.is_equal,
                    )
                    ps3 = ps1p.tile([P, hid + 1], F32, tag="ps3")
                    for q in range(j):
                        nc.tensor.matmul(
                            ps3[:],
                            lhsT=_mmcast(s_t[:, q, :], MM_SCATTER_F32R),
                            rhs=_mmcast(g[:, q, 0:hid + 1], MM_SCATTER_F32R),
                            start=(q == 0), stop=(q == j - 1),
                        )
                    den = blkp.tile([P, 1], F32, tag="den2")
                    nc.vector.tensor_scalar_max(den[:], ps3[:, hid:hid + 1], 1e-30)
                    rec = blkp.tile([P, 1], F32, tag="rec2")
                    nc.vector.reciprocal(rec[:], den[:])
                    out2 = blkp.tile([P, hid + 1], F32, tag="out2")
                    nc.vector.tensor_tensor(
                        out=out2[:], in0=ps3[:],
                        in1=rec[:].to_broadcast([P, hid + 1]),
                        op=mybir.AluOpType.mult,
                    )
                    nc.vector.tensor_tensor(out=out2[:], in0=out2[:], in1=b2t[:],
                                            op=mybir.AluOpType.add)
                    # mean pooling via one-hot matmul (col hid is the count)
                    psp = ps2p.tile([NGRAPH, hid + 1], F32, tag="psp")
                    nc.tensor.matmul(psp[:],
                                     lhsT=oh_t[:, b * NGRAPH:(b + 1) * NGRAPH],
                                     rhs=out2[:], start=True, stop=True)
                    nc.vector.tensor_tensor(out=acc_sum[:], in0=acc_sum[:],
                                            in1=psp[:], op=mybir.AluOpType.add)
                    # max pooling: masked running max
                    tmx = edgep.tile([P, NGRAPH, hid], F32, tag="tmx")
                    nc.vector.tensor_tensor(
                        out=tmx[:],
                        in0=out2[:, None, 0:hid].to_broadcast([P, NGRAPH, hid]),
                        in1=mask_t[:, b * NGRAPH:(b + 1) * NGRAPH][:, :, None]
                            .to_broadcast([P, NGRAPH, hid]),
                        op=mybir.AluOpType.add,
                    )
                    nc.vector.tensor_tensor(out=acc_max[:], in0=acc_max[:],
                                            in1=tmx[:], op=mybir.AluOpType.max)

            # ================= phase 4: reduce + MLP =================
            with (
                nc.named_scope("p4"),
                tc.tile_pool(name="fin", bufs=1) as finp,
                tc.tile_pool(name="pst4", bufs=2, space="PSUM") as pstp,
                tc.tile_pool(name="ps2_4", bufs=2, space="PSUM") as ps2p,
            ):
                # local max partition-reduce: [128, G, hid] -> maxT [hid, G]
                maxT = finp.tile([hid, NGRAPH], F32)
                for gidx in range(NGRAPH):
                    tpm = pstp.tile([hid, P], F32, tag="t4")
                    nc.tensor.transpose(tpm[:], acc_max[:, gidx, :], ident_t[:])
                    nc.vector.reduce_max(maxT[:, gidx:gidx + 1], tpm[:],
                                         axis=mybir.AxisListType.X)
                # collectives (bounce via DRAM)
                sum_d = dram.tile([NGRAPH, hid + 1], F32)
                max_d = dram.tile([hid, NGRAPH], F32)
                nc.sync.dma_start(sum_d[:], acc_sum[:])
                nc.sync.dma_start(max_d[:], maxT[:])
                nc.gpsimd.collective_compute(
                    "AllReduce", mybir.AluOpType.add,
                    replica_groups=[list(range(NCORES))],
                    ins=[sum_d[:]], outs=[sumr_d[:]],
                )
                nc.gpsimd.collective_compute(
                    "AllReduce", mybir.AluOpType.max,
                    replica_groups=[list(range(NCORES))],
                    ins=[max_d[:]], outs=[maxr_d[:]],
                )
                sum_t = finp.tile([NGRAPH, hid + 1], F32)
                nc.sync.dma_start(sum_t[:], sumr_d[:])
                maxR_raw = finp.tile([hid, NGRAPH], F32)
                nc.sync.dma_start(maxR_raw[:], maxr_d[:])
                maxR = finp.tile([hid, NGRAPH], F32)
                nc.vector.tensor_copy(maxR[:], maxR_raw[:])

                cnt = finp.tile([NGRAPH, 1], F32)
                nc.vector.tensor_scalar_max(cnt[:], sum_t[:, hid:hid + 1], 1.0)
                rc = finp.tile([NGRAPH, 1], F32)
                nc.vector.reciprocal(rc[:], cnt[:])
                gp = finp.tile([NGRAPH, 2 * hid], F32)
                nc.vector.tensor_scalar(out=gp[:, 0:hid], in0=sum_t[:, 0:hid],
                                        scalar1=rc[:], scalar2=None,
                                        op0=mybir.AluOpType.mult)
                tpg = pstp.tile([NGRAPH, hid], F32, tag="t4")
                nc.tensor.transpose(tpg[:], maxR[:], ident_t[:hid, :hid])
                nc.vector.tensor_copy(gp[:, hid:2 * hid], tpg[:])

                # classifier MLP (replicated on every core)
                cw1_raw = finp.tile([2 * hid, hid], F32)
                nc.sync.dma_start(cw1_raw[:], cw1_in[:])
                cw1_t = finp.tile([2 * hid, hid], F32)
                nc.vector.tensor_copy(cw1_t[:], cw1_raw[:])
                cw2_raw = finp.tile([hid, 32], F32)
                nc.sync.dma_start(cw2_raw[:], cw2_in[:])
                cw2_t = finp.tile([hid, 32], F32)
                nc.vector.tensor_copy(cw2_t[:], cw2_raw[:])
                cw3_raw = finp.tile([32, 2], F32)
                nc.sync.dma_start(cw3_raw[:], cw3_in[:])
                cw3_t = finp.tile([32, 2], F32)
                nc.vector.tensor_copy(cw3_t[:], cw3_raw[:])
                cb1_t = finp.tile([NGRAPH, hid], F32)
                nc.sync.dma_start(cb1_t[:], cb1t_in[:])
                cb2_t = finp.tile([NGRAPH, 32], F32)
                nc.sync.dma_start(cb2_t[:], cb2t_in[:])
                cb3_t = finp.tile([NGRAPH, 2], F32)
                nc.sync.dma_start(cb3_t[:], cb3t_in[:])

                def _mlp_layer(inp, w_t, b_t, width, relu, tag):
                    # inp [G, K] -> out [G, width]
                    kdim = inp.shape[1]
                    tpp = pstp.tile([kdim, NGRAPH], F32, tag="t4")
                    nc.tensor.transpose(tpp[:], inp[:],
                                        ident_t[:NGRAPH, :NGRAPH])
                    tsb = finp.tile([kdim, NGRAPH], F32)
                    nc.vector.tensor_copy(tsb[:], tpp[:])
                    pso = ps2p.tile([NGRAPH, width], F32, tag="ps4")
                    nc.tensor.matmul(pso[:], lhsT=tsb[:], rhs=w_t[:],
                                     start=True, stop=True)
                    o = finp.tile([NGRAPH, width], F32)
                    nc.vector.tensor_tensor(out=o[:], in0=pso[:], in1=b_t[:],
                                            op=mybir.AluOpType.add)
                    if relu:
                        nc.vector.tensor_scalar_max(o[:], o[:], 0.0)
                    return o

                o1 = _mlp_layer(gp[:], cw1_t[:], cb1_t[:], hid, True, "m1")
                o2 = _mlp_layer(o1[:], cw2_t[:], cb2_t[:], 32, True, "m2")
                o3 = _mlp_layer(o2[:], cw3_t[:], cb3_t[:], 2, False, "m3")
                nc.sync.dma_start(out_ext[:], o3[:])

    if SPLIT_WAITS:
        _split_waits(nc)
    nc.finalize()
    return nc




def _split_waits(nc):
    """Walrus (this toolchain) allows at most 1 sync-wait on engine
    instructions (matmul LDWEIGHTS and some DMA structs reject 2+).
    Hoist excess waits onto fresh sequencer NOPs placed immediately before
    the waiter on the same engine queue: blocking semantics are identical."""
    SKIP = {"InstEventSemaphore", "InstUnconditionalBranch",
            "InstCall", "InstRegisterMove", "InstISA"}
    LIMITS = {"InstMatmult": 1, "InstDMACopy": 1, "InstDrain": 1,
              "InstCollectiveCompute": 1}
    nop_op = nc.isa.Opcode.NEURON_ISA_TPB_OPCODE_NOP
    for bb in nc.main_func.blocks:
        out = []
        for ins in bb.instructions:
            si = ins.sync_info
            tname = type(ins).__name__
            lim = LIMITS.get(tname, 1)
            if (si is not None and tname not in SKIP and si.on_wait
                    and len(si.on_wait) > lim):
                waits = list(si.on_wait)
                excess, keep = waits[:-lim], waits[-lim:]
                for w in excess:
                    eng = nc.engines[ins.engine]
                    nop = eng._isa(nop_op, {})
                    nop.engine = ins.engine
                    nop.sync_info = mybir.SyncInfo(on_wait=[w], on_update=[])
                    nc.inst_map[nop.name] = nop
                    out.append(nop)
                si.on_wait = keep
            out.append(ins)
        bb.instructions[:] = out


# ----------------------------------------------------------------------------
# entry point
# ----------------------------------------------------------------------------

def _make_in_maps(meta, shared, per_core, inputs):
    hid = meta["hid"]
    b1 = np.asarray(inputs["b1"], np.float32)
    b2 = np.asarray(inputs["b2"], np.float32)
    b2t = np.zeros((P, hid + 1), np.float32)
    b2t[:, :hid] = b2[None, :]
    base = dict(
        xT=shared["xT"], W1ext=shared["W1ext"], W2ext=shared["W2ext"],
        pad1=shared["pad1"], pad2=shared["pad2"],
        iota=shared["iota"], ident=shared["ident"],
        b1t=np.broadcast_to(b1, (P, meta["c1"])).copy(),
        b2t=b2t,
        cW1=np.asarray(inputs["cW1"], np.float32),
        cW2=np.asarray(inputs["cW2"], np.float32),
        cW3=np.asarray(inputs["cW3"], np.float32),
        cb1t=np.broadcast_to(np.asarray(inputs["cb1"], np.float32),
                             (NGRAPH, hid)).copy(),
        cb2t=np.broadcast_to(np.asarray(inputs["cb2"], np.float32),
                             (NGRAPH, 32)).copy(),
        cb3t=np.broadcast_to(np.asarray(inputs["cb3"], np.float32),
                             (NGRAPH, 2)).copy(),
    )
    in_maps = []
    for c in range(NCORES):
        m = dict(base)
        m["eidx_src"] = per_core["eidx_src"][c]
        m["eidx_dst"] = per_core["eidx_dst"][c]
        m["slot"] = per_core["slot"][c]
        m["onehot"] = per_core["onehot"][c]
        m["mask"] = per_core["mask"][c]
        in_maps.append(m)
    return in_maps


def kernel(x, edge_index, batch,
           W1, a_src1, a_dst1, b1,
           W2, a_src2, a_dst2, b2,
           cW1, cb1, cW2, cb2, cW3, cb3):
    x = np.asarray(x, np.float32)
    edge_index = np.asarray(edge_index)
    batch = np.asarray(batch)
    meta, shared, per_core = _host_prep(
        x, edge_index, batch,
        np.asarray(W1, np.float32), np.asarray(a_src1, np.float32),
        np.asarray(a_dst1, np.float32), np.asarray(W2, np.float32),
        np.asarray(a_src2, np.float32), np.asarray(a_dst2, np.float32))

    nc = _build_program(meta, b1, b2, cW1, cb1, cW2, cb2, cW3, cb3)
    inputs = dict(b1=b1, b2=b2, cW1=cW1, cb1=cb1, cW2=cW2, cb2=cb2,
                  cW3=cW3, cb3=cb3)
    in_maps = _make_in_maps(meta, shared, per_core, inputs)

    res = run_bass_kernel_spmd(nc, in_maps, list(range(NCORES)))
    return res.results[0]["out"].astype(np.float32)

